# revision 1
# baseline (speedup 1.0000x reference)
"""Trainium2 Bass kernel for nn_CIFARDiffusionLayer.

The reference applies, per channel c, three ADI steps; each step is an
x-sweep (constant-coefficient tridiagonal solve along W), a y-sweep
(same along H), and a multiply by diag(channel_coupling)[c].  Every
sweep is a fixed linear map: solving T x = d with the reference's exact
Thomas recurrence is x = T^{-1} d, and T^{-1} is a dense 256x256 matrix
that depends only on (channel, step, direction).  X-sweeps act on U by
right-multiplication and y-sweeps by left-multiplication, so they all
commute across steps and the whole layer collapses to

    out[b, c] = A_c @ u[b, c] @ B_c
    A_c = s_c^3 * My(c,2) @ My(c,1) @ My(c,0)      (s_c = coupling diag)
    B_c = Mx(c,0)^T @ Mx(c,1)^T @ Mx(c,2)^T

with the tiny 256x256 matrices computed on the host in float64 from the
reference's exact recurrences (including its eps quirks).  The device
work is two 256x256x256 matmuls per (batch, channel) slab, run as
fp32r (full-rate) TensorE matmuls with the data slab as the stationary
operand so each matmul also transposes the slab back and forth.

Sharding: data parallelism over (batch, channel) slabs: 384 slabs are
dealt to 8 cores as 48 generic slabs each (32 of one channel + 16 of
another, per the ASSIGN table), so each core loads only the 2 matrix
pairs it needs (1.0MB instead of 1.5MB of constants) while the NEFF
stays identical across cores.
"""

import sys

if "/opt/trn_rl_repo" not in sys.path:
    sys.path.insert(0, "/opt/trn_rl_repo")

import numpy as np

DT = 0.05
DX = 1.0
NUM_STEPS = 3
EPS = 1e-6
MAX_COEFF = 1.0

N_CORES = 8
B, C, S = 128, 3, 256
B_LOC = B // N_CORES
N_SLAB = 48          # (batch, channel) slabs per core
N_GROUP = N_SLAB // 3
# Per core: ((channel of the 32-slab block, batch start), (channel of the
# 16-slab block, batch start)).  Covers each (b, c) exactly once:
# c0 = 4x32, c1 = 2x32 + 4x16, c2 = 2x32 + 4x16.
ASSIGN = [
    ((0, 0), (1, 64)),
    ((0, 32), (1, 80)),
    ((0, 64), (1, 96)),
    ((0, 96), (1, 112)),
    ((1, 0), (2, 64)),
    ((1, 32), (2, 80)),
    ((2, 0), (2, 96)),
    ((2, 32), (2, 112)),
]


def _core_slab_indices(k):
    (c32, b32), (c16, b16) = ASSIGN[k]
    b_idx = list(range(b32, b32 + 32)) + list(range(b16, b16 + 16))
    c_idx = [c32] * 32 + [c16] * 16
    return b_idx, c_idx


def _thomas_inv(r: float, n: int = S, eps: float = EPS) -> np.ndarray:
    """T^{-1} for the reference's constant-coefficient Thomas solve.

    Mirrors reference._thomas_const exactly (b[0]+eps on the first
    denominator, clamp(min=eps) on interior denominators), evaluated in
    float64 on the identity RHS so columns are T^{-1} e_j.
    """
    a = -r
    b = np.full(n, 1.0 + 2.0 * r, dtype=np.float64)
    b[0] = b[-1] = 1.0 + r
    denom = np.empty(n, dtype=np.float64)
    cp = np.empty(n, dtype=np.float64)
    denom[0] = b[0] + eps
    cp[0] = a / denom[0]
    for i in range(1, n):
        denom[i] = max(b[i] - a * cp[i - 1], eps)
        cp[i] = a / denom[i]
    dp = np.zeros((n, n), dtype=np.float64)
    eye = np.eye(n, dtype=np.float64)
    dp[0] = eye[0] / denom[0]
    for i in range(1, n):
        dp[i] = (eye[i] - a * dp[i - 1]) / denom[i]
    x = np.zeros((n, n), dtype=np.float64)
    x[n - 1] = dp[n - 1]
    for i in range(n - 2, -1, -1):
        x[i] = dp[i] - cp[i] * x[i + 1]
    return x


def _host_mats(alpha_base, beta_base, alpha_spatial, beta_spatial, channel_coupling):
    """mats[c, 0] = A_c^T, mats[c, 1] = B_c, as float32 [C, 2, S, S]."""
    diag = np.diagonal(np.asarray(channel_coupling)).astype(np.float64)
    mats = np.empty((C, 2, S, S), dtype=np.float32)
    for c in range(C):
        am = float(np.mean(np.asarray(alpha_spatial[c], dtype=np.float64)))
        bm = float(np.mean(np.asarray(beta_spatial[c], dtype=np.float64)))
        a_c = np.eye(S, dtype=np.float64)
        b_c = np.eye(S, dtype=np.float64)
        for step in range(NUM_STEPS):
            t = step * DT
            alpha_t = min(max(float(alpha_base[c]) + am * t, EPS), MAX_COEFF)
            beta_t = min(max(float(beta_base[c]) + bm * t, EPS), MAX_COEFF)
            r_a = alpha_t * (DT / 2.0) / DX**2
            r_b = beta_t * (DT / 2.0) / DX**2
            a_c = _thomas_inv(r_b) @ a_c
            b_c = b_c @ _thomas_inv(r_a).T
        mats[c, 0] = (diag[c] ** 3 * a_c).T.astype(np.float32)
        mats[c, 1] = b_c.astype(np.float32)
    return mats


def build_module(repeat: int = 1):
    """Per-core Bass module: out[b,c] = A_c @ u[b,c] @ B_c for 16 slabs x 3 ch.

    repeat > 1 wraps the batch loop in a hardware For_i that re-runs the
    whole kernel body; only used by the timing harness (wall-clock slope
    between two repeat counts isolates the per-iteration device time).
    """
    import concourse.bacc as bacc
    import concourse.tile as tile
    from concourse import mybir

    f32, f32r = mybir.dt.float32, mybir.dt.float32r
    nc = bacc.Bacc(
        "TRN2",
        target_bir_lowering=False,
        debug=False,
        enable_asserts=False,
        num_devices=N_CORES,
    )
    u_d = nc.dram_tensor("u", [N_SLAB, S, S], f32r, kind="ExternalInput")
    m_d = nc.dram_tensor("mats", [2, 2, S, S], f32r, kind="ExternalInput")
    o_d = nc.dram_tensor("out", [N_SLAB, S, S], f32, kind="ExternalOutput")

    with tile.TileContext(nc) as tc:
        with (
            tc.tile_pool(name="consts", bufs=1) as cpool,
            tc.tile_pool(name="ld", bufs=5) as ldpool,
            tc.tile_pool(name="vt", bufs=3) as vtpool,
            tc.tile_pool(name="zs", bufs=4) as zspool,
            tc.tile_pool(name="pv", bufs=2, space="PSUM") as pvpool,
            tc.tile_pool(name="pz", bufs=2, space="PSUM") as pzpool,
        ):
            # Matrix pair q in {0,1}; one [128, 512] tile per (pair, side):
            # [:, 0:256] = k-tile rows 0..127, [:, 256:512] = rows 128..255.
            a_t, b_t = [], []
            for q in range(2):
                at = cpool.tile([128, 512], f32r, tag=f"a{q}")
                nc.sync.dma_start(at[:], m_d[q, 0].rearrange("(k p) w -> p k w", p=128))
                a_t.append(at)
                bt = cpool.tile([128, 512], f32r, tag=f"b{q}")
                nc.sync.dma_start(bt[:], m_d[q, 1].rearrange("(k p) w -> p k w", p=128))
                b_t.append(bt)

            def batch_loop():
                for g in range(N_GROUP):
                    _emit_group(g)

            def _emit_group(g):
                # Load 3 slabs: free layout j*512 + k*256 + w, partition = h%128.
                # Per-slab DMAs keep the SP queue from head-of-line blocking.
                ld = ldpool.tile([128, 3 * 512], f32r)
                for j in range(3):
                    nc.sync.dma_start(
                        ld[:, j * 512 : (j + 1) * 512],
                        u_d[3 * g + j].rearrange("(k p) w -> p k w", p=128),
                    )
                zs = zspool.tile([128, 3 * 512], f32)
                for j in range(3):
                    slab = 3 * g + j
                    q = 0 if slab < 32 else 1
                    base = j * 512
                    # MM1: V^T[w, h'] = sum_h U[h, w] * A^T[h, h']  (data stationary)
                    pv = pvpool.tile([128, 512], f32)
                    for mi in range(2):
                        for k in range(2):
                            nc.tensor.matmul(
                                pv[:, mi * 256 : (mi + 1) * 256],
                                ld[:, base + k * 256 + mi * 128 : base + k * 256 + mi * 128 + 128],
                                a_t[q][:, k * 256 : (k + 1) * 256],
                                start=(k == 0),
                                stop=(k == 1),
                            )
                    vt = vtpool.tile([128, 512], f32r)
                    nc.vector.tensor_copy(vt[:], pv[:])
                    # MM2: Z[h', w'] = sum_w V^T[w, h'] * B[w, w']
                    pz = pzpool.tile([128, 512], f32)
                    for mi in range(2):
                        for k in range(2):
                            nc.tensor.matmul(
                                pz[:, mi * 256 : (mi + 1) * 256],
                                vt[:, k * 256 + mi * 128 : k * 256 + mi * 128 + 128],
                                b_t[q][:, k * 256 : (k + 1) * 256],
                                start=(k == 0),
                                stop=(k == 1),
                            )
                    nc.scalar.copy(zs[:, base : base + 512], pz[:])
                # Out-DMA on the ACT HWDGE ring: keeps the SP queue free for
                # input loads (out-DMAs wait on compute; SP head-of-line
                # blocking would stall the next group's loads behind them).
                nc.scalar.dma_start(
                    o_d[3 * g : 3 * g + 3].rearrange("s (k p) w -> p s k w", p=128),
                    zs[:],
                )

            if repeat == 1:
                batch_loop()
            else:
                # staggered_reset avoids the ~3us all-engine barrier at the
                # loop back-edge, so the slope measurement better matches the
                # barrier-free single-shot kernel.
                with tc.For_i(0, repeat, 1, staggered_reset=True):
                    batch_loop()
    nc.compile()
    return nc


_CACHE = {}


def _axon_runner():
    """Build (once) a jitted 8-way sharded executor for the axon/PJRT path.

    Mirrors concourse.bass2jax.run_bass_via_pjrt but keeps the compiled
    executable alive so repeat kernel() calls skip retracing + NEFF
    recompilation.
    """
    if "runner" in _CACHE:
        return _CACHE["runner"]
    import jax
    from jax.experimental.shard_map import shard_map
    from jax.sharding import Mesh, NamedSharding, PartitionSpec

    from concourse import bass2jax, mybir

    nc = build_module()
    bass2jax.install_neuronx_cc_hook()
    partition_name = nc.partition_id_tensor.name if nc.partition_id_tensor else None
    in_names, out_names, out_avals = [], [], []
    for alloc in nc.m.functions[0].allocations:
        if not isinstance(alloc, mybir.MemoryLocationSet):
            continue
        name = alloc.memorylocations[0].name
        if alloc.kind == "ExternalInput":
            if name != partition_name:
                in_names.append(name)
        elif alloc.kind == "ExternalOutput":
            out_names.append(name)
            out_avals.append(
                jax.core.ShapedArray(tuple(alloc.tensor_shape), mybir.dt.np(alloc.dtype))
            )
    n_params = len(in_names)
    n_outs = len(out_avals)
    all_names = in_names + out_names + ([partition_name] if partition_name else [])
    donate = tuple(range(n_params, n_params + n_outs))

    def _body(*args):
        operands = list(args)
        if partition_name is not None:
            operands.append(bass2jax.partition_id_tensor())
        return tuple(
            bass2jax._bass_exec_p.bind(
                *operands,
                out_avals=tuple(out_avals),
                in_names=tuple(all_names),
                out_names=tuple(out_names),
                lowering_input_output_aliases=(),
                sim_require_finite=True,
                sim_require_nnan=True,
                nc=nc,
            )
        )

    devices = jax.devices()[:N_CORES]
    mesh = Mesh(np.asarray(devices), ("core",))
    spec = NamedSharding(mesh, PartitionSpec("core"))
    sharded = jax.jit(
        shard_map(
            _body,
            mesh=mesh,
            in_specs=(PartitionSpec("core"),) * (n_params + n_outs),
            out_specs=(PartitionSpec("core"),) * n_outs,
            check_rep=False,
        ),
        donate_argnums=donate,
        keep_unused=True,
    )

    def run(u_cores, mats_cores):
        per_core = {
            "u": np.concatenate(u_cores, axis=0),
            "mats": np.concatenate(mats_cores, axis=0),
        }
        xs = [jax.device_put(per_core[nm], spec) for nm in in_names]
        zs = [
            jax.device_put(
                np.zeros((N_CORES * a.shape[0], *a.shape[1:]), a.dtype), spec
            )
            for a in out_avals
        ]
        outs = sharded(*xs, *zs)
        out = np.asarray(outs[out_names.index("out")])
        return out.reshape(N_CORES, N_SLAB, S, S)

    _CACHE["runner"] = run
    return run


def kernel(u, alpha_base, beta_base, alpha_spatial, beta_spatial, channel_coupling):
    from concourse._compat import axon_active

    u = np.ascontiguousarray(np.asarray(u, dtype=np.float32))
    mats_full = _host_mats(
        np.asarray(alpha_base, dtype=np.float32),
        np.asarray(beta_base, dtype=np.float32),
        np.asarray(alpha_spatial, dtype=np.float32),
        np.asarray(beta_spatial, dtype=np.float32),
        np.asarray(channel_coupling, dtype=np.float32),
    )
    u_cores, mats_cores, idxs = [], [], []
    for k in range(N_CORES):
        b_idx, c_idx = _core_slab_indices(k)
        idxs.append((b_idx, c_idx))
        u_cores.append(np.ascontiguousarray(u[b_idx, c_idx]))
        (c32, _), (c16, _) = ASSIGN[k]
        mats_cores.append(np.stack([mats_full[c32], mats_full[c16]]))

    if axon_active():
        res = _axon_runner()(u_cores, mats_cores)
    else:
        # Native path (/dev/neuron* present): run via NRT on cores 0-7.
        from concourse.bass_utils import run_bass_kernel_spmd

        nc = _CACHE.setdefault("nc", build_module())
        in_maps = [
            {"u": u_cores[k], "mats": mats_cores[k]} for k in range(N_CORES)
        ]
        rr = run_bass_kernel_spmd(nc, in_maps, core_ids=list(range(N_CORES)))
        res = np.stack([r["out"] for r in rr.results])

    out = np.empty((B, C, S, S), dtype=np.float32)
    for k in range(N_CORES):
        b_idx, c_idx = idxs[k]
        out[b_idx, c_idx] = res[k]
    return out



# revision 6
# speedup vs baseline: 1.5637x; 1.5637x over previous
"""Trainium2 Bass kernel for nn_CIFARDiffusionLayer.

The reference applies, per channel c, three ADI steps; each step is an
x-sweep (constant-coefficient tridiagonal solve along W), a y-sweep
(same along H), and a multiply by diag(channel_coupling)[c].  Every
sweep is a fixed linear map, so the whole layer collapses to

    out[b, c] = A_c @ u[b, c] @ B_c
    A_c = s_c^3 * My(c,2) @ My(c,1) @ My(c,0)      (s_c = coupling diag)
    B_c = Mx(c,0)^T @ Mx(c,1)^T @ Mx(c,2)^T

with the 256x256 matrices computed on the host in float64 from the
reference's exact recurrences (including its eps quirks).

Device-side optimizations over a dense-fp32 version:
  * bf16 end-to-end (input, matrices, intermediate, output).  HBM
    traffic halves to ~12.6 MB/core, which is the roofline for this
    memory-bound problem; bf16 rounding contributes ~3e-3 relative
    error against a 2e-2 budget.
  * The matrices are inverses of tridiagonal systems with r~0.0075, so
    their entries decay by ~2e-2 per step off the diagonal: beyond
    |i-j| >= 16 they are exactly zero in fp32.  Each 256-column matmul
    contraction is split into disjoint output regions [0,112) (k-block
    0 only), [112,144) (both blocks), [144,256) (block 1 only), cutting
    PE streaming from 1024 to 576 columns per slab-side.
  * Host pre-packs inputs into the exact SBUF layout ([partition, k, w]
    interleave) so every DMA is a flat 128-partition transfer with
    multi-KB contiguous runs per partition.

Sharding: data parallelism over (batch, channel) slabs: 384 slabs are
dealt to 8 cores as 48 slabs each (32 of one channel + 16 of another,
per the ASSIGN table), so each core loads only the 2 matrix pairs it
needs while the NEFF stays identical across cores.
"""

import sys

if "/opt/trn_rl_repo" not in sys.path:
    sys.path.insert(0, "/opt/trn_rl_repo")

import numpy as np
import ml_dtypes

BF16 = ml_dtypes.bfloat16

DT = 0.05
DX = 1.0
NUM_STEPS = 3
EPS = 1e-6
MAX_COEFF = 1.0

N_CORES = 8
B, C, S = 128, 3, 256
N_SLAB = 48          # (batch, channel) slabs per core
G = 4                # slabs per DMA chunk
NCHUNK = N_SLAB // G
# Banded contraction: output regions and which 128-row k-blocks feed them.
REGIONS = [(0, 112, (0,)), (112, 144, (0, 1)), (144, 256, (1,))]

# Per core: ((channel of the 32-slab block, batch start), (channel of the
# 16-slab block, batch start)).  Covers each (b, c) exactly once:
# c0 = 4x32, c1 = 2x32 + 4x16, c2 = 2x32 + 4x16.
ASSIGN = [
    ((0, 0), (1, 64)),
    ((0, 32), (1, 80)),
    ((0, 64), (1, 96)),
    ((0, 96), (1, 112)),
    ((1, 0), (2, 64)),
    ((1, 32), (2, 80)),
    ((2, 0), (2, 96)),
    ((2, 32), (2, 112)),
]


def _core_slab_indices(k):
    (c32, b32), (c16, b16) = ASSIGN[k]
    b_idx = list(range(b32, b32 + 32)) + list(range(b16, b16 + 16))
    c_idx = [c32] * 32 + [c16] * 16
    return b_idx, c_idx


def _thomas_inv(r: float, n: int = S, eps: float = EPS) -> np.ndarray:
    """T^{-1} for the reference's constant-coefficient Thomas solve.

    Mirrors reference._thomas_const exactly (b[0]+eps on the first
    denominator, clamp(min=eps) on interior denominators), evaluated in
    float64 on the identity RHS so columns are T^{-1} e_j.
    """
    a = -r
    b = np.full(n, 1.0 + 2.0 * r, dtype=np.float64)
    b[0] = b[-1] = 1.0 + r
    denom = np.empty(n, dtype=np.float64)
    cp = np.empty(n, dtype=np.float64)
    denom[0] = b[0] + eps
    cp[0] = a / denom[0]
    for i in range(1, n):
        denom[i] = max(b[i] - a * cp[i - 1], eps)
        cp[i] = a / denom[i]
    dp = np.zeros((n, n), dtype=np.float64)
    eye = np.eye(n, dtype=np.float64)
    dp[0] = eye[0] / denom[0]
    for i in range(1, n):
        dp[i] = (eye[i] - a * dp[i - 1]) / denom[i]
    x = np.zeros((n, n), dtype=np.float64)
    x[n - 1] = dp[n - 1]
    for i in range(n - 2, -1, -1):
        x[i] = dp[i] - cp[i] * x[i + 1]
    return x


def _host_mats(alpha_base, beta_base, alpha_spatial, beta_spatial, channel_coupling):
    """mats[c, 0] = A_c^T, mats[c, 1] = B_c, as float32 [C, 2, S, S]."""
    diag = np.diagonal(np.asarray(channel_coupling)).astype(np.float64)
    mats = np.empty((C, 2, S, S), dtype=np.float32)
    for c in range(C):
        am = float(np.mean(np.asarray(alpha_spatial[c], dtype=np.float64)))
        bm = float(np.mean(np.asarray(beta_spatial[c], dtype=np.float64)))
        a_c = np.eye(S, dtype=np.float64)
        b_c = np.eye(S, dtype=np.float64)
        for step in range(NUM_STEPS):
            t = step * DT
            alpha_t = min(max(float(alpha_base[c]) + am * t, EPS), MAX_COEFF)
            beta_t = min(max(float(beta_base[c]) + bm * t, EPS), MAX_COEFF)
            r_a = alpha_t * (DT / 2.0) / DX**2
            r_b = beta_t * (DT / 2.0) / DX**2
            a_c = _thomas_inv(r_b) @ a_c
            b_c = b_c @ _thomas_inv(r_a).T
        mats[c, 0] = (diag[c] ** 3 * a_c).T.astype(np.float32)
        mats[c, 1] = b_c.astype(np.float32)
    return mats


def _pack_u_core(u_core):
    """[48, 256, 256] f32 -> [NCHUNK, 128, G*512] bf16 in [p, g, k, w] order."""
    x = u_core.astype(BF16).reshape(NCHUNK, G, 2, 128, 256)
    x = np.ascontiguousarray(x.transpose(0, 3, 1, 2, 4))
    return x.reshape(NCHUNK, 128, G * 512)


def _pack_mats_core(mats_core):
    """[2, 2, 256, 256] f32 -> [2, 2, 128, 512] bf16 in [p, k, col] order."""
    m = mats_core.astype(BF16).reshape(2, 2, 2, 128, 256)
    m = np.ascontiguousarray(m.transpose(0, 1, 3, 2, 4))
    return m.reshape(2, 2, 128, 512)


def _unpack_out_core(r):
    """[NCHUNK, 128, G*512] bf16 -> [48, 256, 256] f32."""
    x = r.reshape(NCHUNK, 128, G, 2, 256).transpose(0, 2, 3, 1, 4)
    return np.ascontiguousarray(x).reshape(N_SLAB, S, S).astype(np.float32)


def prep_core_arrays(u, mats_full):
    """Returns (u_cores, mats_cores, idxs): packed per-core device inputs."""
    u_cores, mats_cores, idxs = [], [], []
    for k in range(N_CORES):
        b_idx, c_idx = _core_slab_indices(k)
        idxs.append((b_idx, c_idx))
        u_cores.append(_pack_u_core(np.ascontiguousarray(u[b_idx, c_idx])))
        (c32, _), (c16, _) = ASSIGN[k]
        mats_cores.append(
            _pack_mats_core(np.stack([mats_full[c32], mats_full[c16]]))
        )
    return u_cores, mats_cores, idxs


def build_module(repeat: int = 1):
    """Per-core Bass module: out[b,c] = A_c @ u[b,c] @ B_c for 48 slabs.

    repeat > 1 wraps the batch loop in a hardware For_i that re-runs the
    whole kernel body; only used by the timing harness (wall-clock slope
    between two repeat counts isolates the per-iteration device time).
    """
    import concourse.bacc as bacc
    import concourse.tile as tile
    from concourse import mybir

    f32, bf16 = mybir.dt.float32, mybir.dt.bfloat16
    nc = bacc.Bacc(
        "TRN2",
        target_bir_lowering=False,
        debug=False,
        enable_asserts=False,
        num_devices=N_CORES,
    )
    u_d = nc.dram_tensor("u", [NCHUNK, 128, G * 512], bf16, kind="ExternalInput")
    m_d = nc.dram_tensor("mats", [2, 2, 128, 512], bf16, kind="ExternalInput")
    o_d = nc.dram_tensor("out", [NCHUNK, 128, G * 512], bf16, kind="ExternalOutput")

    with tile.TileContext(nc) as tc:
        with (
            tc.tile_pool(name="consts", bufs=1) as cpool,
            tc.tile_pool(name="ld", bufs=3) as ldpool,
            tc.tile_pool(name="vt", bufs=6) as vtpool,
            tc.tile_pool(name="zs", bufs=3) as zspool,
            tc.tile_pool(name="pv", bufs=4, space="PSUM") as pvpool,
            tc.tile_pool(name="pz", bufs=4, space="PSUM") as pzpool,
        ):
            # Matrix pair q in {0,1}; [128, 512] = [h%128, (h//128)*256 + col].
            # Const loads ride the ACT HWDGE ring, which is otherwise idle at
            # start, so the first input chunk on the SP ring is not delayed.
            a_t, b_t = [], []
            for q in range(2):
                at = cpool.tile([128, 512], bf16, tag=f"a{q}")
                nc.scalar.dma_start(at[:], m_d[q, 0])
                a_t.append(at)
                bt = cpool.tile([128, 512], bf16, tag=f"b{q}")
                nc.scalar.dma_start(bt[:], m_d[q, 1])
                b_t.append(bt)

            def emit_mm(out_ps, lhs_of, rhs_t, q):
                """One 256x256x256 product as banded region matmuls.

                lhs_of(mi, kb) yields the stationary [128,128] data tile;
                rhs_t is the [128, 512] matrix tile ([row%128, kb*256+col]).
                """
                for mi in range(2):
                    for kb in range(2):
                        lhsT = lhs_of(mi, kb)
                        for r0, r1, kbs in REGIONS:
                            if kb not in kbs:
                                continue
                            nc.tensor.matmul(
                                out_ps[:, mi * 256 + r0 : mi * 256 + r1],
                                lhsT,
                                rhs_t[:, kb * 256 + r0 : kb * 256 + r1],
                                start=(kb == kbs[0]),
                                stop=(kb == kbs[-1]),
                            )

            # Two-stage software pipeline with a DEPTH-slab stagger: MM2 for
            # slab i is emitted after MM1 for slab i+DEPTH.  PE executes in
            # program order, so without the stagger it would idle between
            # MM1(i) and MM2(i) while VectorE drains pv(i) into vt(i); with
            # it, PE streams MM1s of later slabs during every copy.
            DEPTH = 4

            def batch_loop():
                pending = []  # (vt, q, zs, base, ch, last_in_chunk)

                def flush_one():
                    vt, q, zs, base, ch, last = pending.pop(0)
                    # MM2: Z[h', w'] = sum_w V^T[w, h'] * B[w, w']
                    pz = pzpool.tile([128, 512], f32)
                    emit_mm(
                        pz,
                        lambda mi, kb: vt[
                            :, kb * 256 + mi * 128 : kb * 256 + mi * 128 + 128
                        ],
                        b_t[q],
                        q,
                    )
                    nc.scalar.copy(zs[:, base : base + 512], pz[:])
                    if last:
                        # Out-DMA on the ACT HWDGE ring keeps the SP queue
                        # free for input loads.
                        nc.scalar.dma_start(o_d[ch], zs[:])

                for ch in range(NCHUNK):
                    ld = ldpool.tile([128, G * 512], bf16)
                    nc.sync.dma_start(ld[:], u_d[ch])
                    zs = zspool.tile([128, G * 512], bf16)
                    for g in range(G):
                        slab = ch * G + g
                        q = 0 if slab < 32 else 1
                        base = g * 512
                        # MM1: V^T[w, h'] = sum_h U[h, w] * A^T[h, h']
                        pv = pvpool.tile([128, 512], f32)
                        emit_mm(
                            pv,
                            lambda mi, kb, base=base, ld=ld: ld[
                                :,
                                base + kb * 256 + mi * 128 : base + kb * 256 + mi * 128 + 128,
                            ],
                            a_t[q],
                            q,
                        )
                        vt = vtpool.tile([128, 512], bf16)
                        nc.vector.tensor_copy(vt[:], pv[:])
                        pending.append((vt, q, zs, base, ch, g == G - 1))
                        while len(pending) > DEPTH:
                            flush_one()
                while pending:
                    flush_one()

            if repeat == 1:
                batch_loop()
            else:
                # staggered_reset avoids the ~3us all-engine barrier at the
                # loop back-edge, so the slope measurement better matches the
                # barrier-free single-shot kernel.
                with tc.For_i(0, repeat, 1, staggered_reset=True):
                    batch_loop()
    nc.compile()
    return nc


_CACHE = {}


def _axon_runner():
    """Build (once) a jitted 8-way sharded executor for the axon/PJRT path.

    Mirrors concourse.bass2jax.run_bass_via_pjrt but keeps the compiled
    executable alive so repeat kernel() calls skip retracing + NEFF
    recompilation.
    """
    if "runner" in _CACHE:
        return _CACHE["runner"]
    import jax
    from jax.experimental.shard_map import shard_map
    from jax.sharding import Mesh, NamedSharding, PartitionSpec

    from concourse import bass2jax, mybir

    nc = build_module()
    bass2jax.install_neuronx_cc_hook()
    partition_name = nc.partition_id_tensor.name if nc.partition_id_tensor else None
    in_names, out_names, out_avals = [], [], []
    for alloc in nc.m.functions[0].allocations:
        if not isinstance(alloc, mybir.MemoryLocationSet):
            continue
        name = alloc.memorylocations[0].name
        if alloc.kind == "ExternalInput":
            if name != partition_name:
                in_names.append(name)
        elif alloc.kind == "ExternalOutput":
            out_names.append(name)
            out_avals.append(
                jax.core.ShapedArray(tuple(alloc.tensor_shape), mybir.dt.np(alloc.dtype))
            )
    n_params = len(in_names)
    n_outs = len(out_avals)
    all_names = in_names + out_names + ([partition_name] if partition_name else [])
    donate = tuple(range(n_params, n_params + n_outs))

    def _body(*args):
        operands = list(args)
        if partition_name is not None:
            operands.append(bass2jax.partition_id_tensor())
        return tuple(
            bass2jax._bass_exec_p.bind(
                *operands,
                out_avals=tuple(out_avals),
                in_names=tuple(all_names),
                out_names=tuple(out_names),
                lowering_input_output_aliases=(),
                sim_require_finite=True,
                sim_require_nnan=True,
                nc=nc,
            )
        )

    devices = jax.devices()[:N_CORES]
    mesh = Mesh(np.asarray(devices), ("core",))
    spec = NamedSharding(mesh, PartitionSpec("core"))
    sharded = jax.jit(
        shard_map(
            _body,
            mesh=mesh,
            in_specs=(PartitionSpec("core"),) * (n_params + n_outs),
            out_specs=(PartitionSpec("core"),) * n_outs,
            check_rep=False,
        ),
        donate_argnums=donate,
        keep_unused=True,
    )

    def run(u_cores, mats_cores):
        per_core = {
            "u": np.concatenate(u_cores, axis=0),
            "mats": np.concatenate(mats_cores, axis=0),
        }
        xs = [jax.device_put(per_core[nm], spec) for nm in in_names]
        zs = [
            jax.device_put(
                np.zeros((N_CORES * a.shape[0], *a.shape[1:]), a.dtype), spec
            )
            for a in out_avals
        ]
        outs = sharded(*xs, *zs)
        out = np.asarray(outs[out_names.index("out")])
        return out.reshape(N_CORES, NCHUNK, 128, G * 512)

    _CACHE["runner"] = run
    return run


def kernel(u, alpha_base, beta_base, alpha_spatial, beta_spatial, channel_coupling):
    from concourse._compat import axon_active

    u = np.ascontiguousarray(np.asarray(u, dtype=np.float32))
    mats_full = _host_mats(
        np.asarray(alpha_base, dtype=np.float32),
        np.asarray(beta_base, dtype=np.float32),
        np.asarray(alpha_spatial, dtype=np.float32),
        np.asarray(beta_spatial, dtype=np.float32),
        np.asarray(channel_coupling, dtype=np.float32),
    )
    u_cores, mats_cores, idxs = prep_core_arrays(u, mats_full)

    if axon_active():
        res = _axon_runner()(u_cores, mats_cores)
    else:
        # Native path (/dev/neuron* present): run via NRT on cores 0-7.
        from concourse.bass_utils import run_bass_kernel_spmd

        nc = _CACHE.setdefault("nc", build_module())
        in_maps = [
            {"u": u_cores[k], "mats": mats_cores[k]} for k in range(N_CORES)
        ]
        rr = run_bass_kernel_spmd(nc, in_maps, core_ids=list(range(N_CORES)))
        res = np.stack([r["out"] for r in rr.results])

    out = np.empty((B, C, S, S), dtype=np.float32)
    for k in range(N_CORES):
        b_idx, c_idx = idxs[k]
        out[b_idx, c_idx] = _unpack_out_core(res[k])
    return out


# revision 23
# speedup vs baseline: 2.0145x; 1.2883x over previous
"""Trainium2 Bass kernel for nn_CIFARDiffusionLayer.

The reference applies, per channel c, three ADI steps; each step is an
x-sweep (constant-coefficient tridiagonal solve along W), a y-sweep
(same along H), and a multiply by diag(channel_coupling)[c].  Every
sweep is a fixed linear map, so the whole layer collapses to

    out[b, c] = A_c @ u[b, c] @ B_c
    A_c = s_c^3 * My(c,2) @ My(c,1) @ My(c,0)      (s_c = coupling diag)
    B_c = Mx(c,0)^T @ Mx(c,1)^T @ Mx(c,2)^T

with the 256x256 matrices computed on the host in float64 from the
reference's exact recurrences (including its eps quirks).

Device-side optimizations over a dense-fp32 version:
  * bf16 end-to-end (input, matrices, intermediate, output).  HBM
    traffic halves to ~12.6 MB/core, which is the roofline for this
    memory-bound problem; bf16 rounding contributes ~3e-3 relative
    error against a 2e-2 budget.
  * The matrices are inverses of tridiagonal systems with r~0.0075, so
    their entries decay by ~2e-2 per step off the diagonal: beyond
    |i-j| >= 16 they are exactly zero in fp32.  Each 256-column matmul
    contraction is split into disjoint output regions [0,112) (k-block
    0 only), [112,144) (both blocks), [144,256) (block 1 only), cutting
    PE streaming from 1024 to 576 columns per slab-side.
  * Host pre-packs inputs into the exact SBUF layout ([partition, k, w]
    interleave) so every DMA is a flat 128-partition transfer with
    multi-KB contiguous runs per partition (1 MB input chunks of 8
    slabs, 512 KB output chunks of 4 slabs).
  * Two-stage software pipeline: MM2 of slab i is emitted after MM1 of
    slab i+4, so TensorE (in-order execution) never stalls behind the
    PSUM->SBUF drain of the intermediate; VectorE handles that drain
    while ScalarE stages outputs.  In/out DMAs both ride the SP HWDGE
    ring (measured faster than ACT-issued out-DMAs, whose sem-wait
    blocks the ACT sequencer between copy bursts).

Sharding: data parallelism over (batch, channel) slabs: 384 slabs are
dealt to 8 cores as 48 slabs each (32 of one channel + 16 of another,
per the ASSIGN table), so each core loads only the 2 matrix pairs it
needs while the NEFF stays identical across cores.
"""

import sys

if "/opt/trn_rl_repo" not in sys.path:
    sys.path.insert(0, "/opt/trn_rl_repo")

import numpy as np
import ml_dtypes

BF16 = ml_dtypes.bfloat16

DT = 0.05
DX = 1.0
NUM_STEPS = 3
EPS = 1e-6
MAX_COEFF = 1.0

N_CORES = 8
B, C, S = 128, 3, 256
N_SLAB = 48          # (batch, channel) slabs per core
G_IN = 8             # slabs per input DMA chunk
G_OUT = 4            # slabs per output DMA chunk
NCH_IN = N_SLAB // G_IN
NCH_OUT = N_SLAB // G_OUT
# Banded contraction: output regions and which 128-row k-blocks feed them.
REGIONS = [(0, 112, (0,)), (112, 144, (0, 1)), (144, 256, (1,))]

# Per core: ((channel of the 32-slab block, batch start), (channel of the
# 16-slab block, batch start)).  Covers each (b, c) exactly once:
# c0 = 4x32, c1 = 2x32 + 4x16, c2 = 2x32 + 4x16.
ASSIGN = [
    ((0, 0), (1, 64)),
    ((0, 32), (1, 80)),
    ((0, 64), (1, 96)),
    ((0, 96), (1, 112)),
    ((1, 0), (2, 64)),
    ((1, 32), (2, 80)),
    ((2, 0), (2, 96)),
    ((2, 32), (2, 112)),
]


def _core_slab_indices(k):
    (c32, b32), (c16, b16) = ASSIGN[k]
    b_idx = list(range(b32, b32 + 32)) + list(range(b16, b16 + 16))
    c_idx = [c32] * 32 + [c16] * 16
    return b_idx, c_idx


def _thomas_inv(r: float, n: int = S, eps: float = EPS) -> np.ndarray:
    """T^{-1} for the reference's constant-coefficient Thomas solve.

    Mirrors reference._thomas_const exactly (b[0]+eps on the first
    denominator, clamp(min=eps) on interior denominators), evaluated in
    float64 on the identity RHS so columns are T^{-1} e_j.
    """
    a = -r
    b = np.full(n, 1.0 + 2.0 * r, dtype=np.float64)
    b[0] = b[-1] = 1.0 + r
    denom = np.empty(n, dtype=np.float64)
    cp = np.empty(n, dtype=np.float64)
    denom[0] = b[0] + eps
    cp[0] = a / denom[0]
    for i in range(1, n):
        denom[i] = max(b[i] - a * cp[i - 1], eps)
        cp[i] = a / denom[i]
    dp = np.zeros((n, n), dtype=np.float64)
    eye = np.eye(n, dtype=np.float64)
    dp[0] = eye[0] / denom[0]
    for i in range(1, n):
        dp[i] = (eye[i] - a * dp[i - 1]) / denom[i]
    x = np.zeros((n, n), dtype=np.float64)
    x[n - 1] = dp[n - 1]
    for i in range(n - 2, -1, -1):
        x[i] = dp[i] - cp[i] * x[i + 1]
    return x


def _host_mats(alpha_base, beta_base, alpha_spatial, beta_spatial, channel_coupling):
    """mats[c, 0] = A_c^T, mats[c, 1] = B_c, as float32 [C, 2, S, S]."""
    diag = np.diagonal(np.asarray(channel_coupling)).astype(np.float64)
    mats = np.empty((C, 2, S, S), dtype=np.float32)
    for c in range(C):
        am = float(np.mean(np.asarray(alpha_spatial[c], dtype=np.float64)))
        bm = float(np.mean(np.asarray(beta_spatial[c], dtype=np.float64)))
        a_c = np.eye(S, dtype=np.float64)
        b_c = np.eye(S, dtype=np.float64)
        for step in range(NUM_STEPS):
            t = step * DT
            alpha_t = min(max(float(alpha_base[c]) + am * t, EPS), MAX_COEFF)
            beta_t = min(max(float(beta_base[c]) + bm * t, EPS), MAX_COEFF)
            r_a = alpha_t * (DT / 2.0) / DX**2
            r_b = beta_t * (DT / 2.0) / DX**2
            a_c = _thomas_inv(r_b) @ a_c
            b_c = b_c @ _thomas_inv(r_a).T
        mats[c, 0] = (diag[c] ** 3 * a_c).T.astype(np.float32)
        mats[c, 1] = b_c.astype(np.float32)
    return mats


INT8_IN = False  # flip after on-device validation of the SWDGE int8->bf16 cast


def _pack_u_core(u_core, int8_in=None):
    """[48, 256, 256] f32 -> [NCH_IN, 128, G_IN*512] device dtype, [p, g, k, w].

    With int8_in, u is symmetrically quantized to int8 (scale returned so the
    host can fold 1/scale into the A matrices); the input DMA upcasts to bf16
    on the fly (SWDGE cast), halving input HBM traffic.
    """
    if int8_in is None:
        int8_in = INT8_IN
    if int8_in:
        # 4-sigma clip beats absmax scaling for gaussian data: finer steps
        # for the bulk at the cost of a tiny tail-clipping term.
        scale = 127.0 / (4.0 * float(u_core.std()) + 1e-30)
        q = np.clip(np.rint(u_core * scale), -127, 127).astype(np.int8)
        x = q.reshape(NCH_IN, G_IN, 2, 128, 256)
    else:
        scale = 1.0
        x = u_core.astype(BF16).reshape(NCH_IN, G_IN, 2, 128, 256)
    x = np.ascontiguousarray(x.transpose(0, 3, 1, 2, 4))
    return x.reshape(NCH_IN, 128, G_IN * 512), scale


def _pack_mats_core(mats_core):
    """[2, 2, 256, 256] f32 -> [2, 2, 128, 512] bf16 in [p, k, col] order."""
    m = mats_core.astype(BF16).reshape(2, 2, 2, 128, 256)
    m = np.ascontiguousarray(m.transpose(0, 1, 3, 2, 4))
    return m.reshape(2, 2, 128, 512)


def _unpack_out_core(r):
    """[NCH_OUT, 128, G_OUT*512] bf16 -> [48, 256, 256] f32."""
    x = r.reshape(NCH_OUT, 128, G_OUT, 2, 256).transpose(0, 2, 3, 1, 4)
    return np.ascontiguousarray(x).reshape(N_SLAB, S, S).astype(np.float32)


def prep_core_arrays(u, mats_full, int8_in=None):
    """Returns (u_cores, mats_cores, idxs): packed per-core device inputs."""
    u_cores, mats_cores, idxs = [], [], []
    for k in range(N_CORES):
        b_idx, c_idx = _core_slab_indices(k)
        idxs.append((b_idx, c_idx))
        uc, scale = _pack_u_core(np.ascontiguousarray(u[b_idx, c_idx]), int8_in)
        u_cores.append(uc)
        (c32, _), (c16, _) = ASSIGN[k]
        mc = np.stack([mats_full[c32], mats_full[c16]])
        if scale != 1.0:
            mc = mc.copy()
            mc[:, 0] /= scale  # fold dequant into A^T
        mats_cores.append(_pack_mats_core(mc))
    return u_cores, mats_cores, idxs


def build_module(repeat: int = 1, **overrides):
    """Per-core Bass module: out[b,c] = A_c @ u[b,c] @ B_c for 48 slabs.

    repeat > 1 wraps the batch loop in a hardware For_i that re-runs the
    whole kernel body; only used by the timing harness (wall-clock slope
    between two repeat counts isolates the per-iteration device time).
    overrides: tuning knobs for the HW sweep harness (see cfg below).
    """
    import concourse.bacc as bacc
    import concourse.tile as tile
    from concourse import mybir

    cfg = dict(
        depth=4, ldb=3, vtb=6, zsb=4, pvb=4, pzb=4, out_eng="sp", in_eng="sp",
        g_out=G_OUT, int8_in=INT8_IN,
    )
    cfg.update(overrides)
    g_out = cfg["g_out"]
    if cfg["int8_in"]:
        cfg["in_eng"] = "gp"  # only SWDGE can cast during DMA

    f32, bf16 = mybir.dt.float32, mybir.dt.bfloat16
    nc = bacc.Bacc(
        "TRN2",
        target_bir_lowering=False,
        debug=False,
        enable_asserts=False,
        num_devices=N_CORES,
    )
    u_dt = mybir.dt.int8 if cfg["int8_in"] else bf16
    u_d = nc.dram_tensor("u", [NCH_IN, 128, G_IN * 512], u_dt, kind="ExternalInput")
    m_d = nc.dram_tensor("mats", [2, 2, 128, 512], bf16, kind="ExternalInput")
    o_d = nc.dram_tensor("out", [N_SLAB // g_out, 128, g_out * 512], bf16, kind="ExternalOutput")

    with tile.TileContext(nc) as tc:
        with (
            tc.tile_pool(name="consts", bufs=1) as cpool,
            tc.tile_pool(name="ld", bufs=cfg["ldb"]) as ldpool,
            tc.tile_pool(name="vt", bufs=cfg["vtb"]) as vtpool,
            tc.tile_pool(name="zs", bufs=cfg["zsb"]) as zspool,
            tc.tile_pool(name="pv", bufs=cfg["pvb"], space="PSUM") as pvpool,
            tc.tile_pool(name="pz", bufs=cfg["pzb"], space="PSUM") as pzpool,
        ):
            engs = {"sp": nc.sync, "act": nc.scalar, "gp": nc.gpsimd}
            out_dma_eng = engs[cfg["out_eng"]]
            in_dma_eng = engs[cfg["in_eng"]]
            # Matrix pair q in {0,1}; [128, 512] = [h%128, (h//128)*256 + col].
            # Const loads ride the ACT HWDGE ring, which is otherwise idle at
            # start, so the first input chunk on the SP ring is not delayed.
            a_t, b_t = [], []
            for q in range(2):
                at = cpool.tile([128, 512], bf16, tag=f"a{q}")
                nc.scalar.dma_start(at[:], m_d[q, 0])
                a_t.append(at)
                bt = cpool.tile([128, 512], bf16, tag=f"b{q}")
                nc.scalar.dma_start(bt[:], m_d[q, 1])
                b_t.append(bt)

            def emit_mm(out_ps, lhs_of, rhs_t):
                """One 256x256x256 product as banded region matmuls.

                lhs_of(mi, kb) yields the stationary [128,128] data tile;
                rhs_t is the [128, 512] matrix tile ([row%128, kb*256+col]).
                """
                for mi in range(2):
                    for kb in range(2):
                        lhsT = lhs_of(mi, kb)
                        for r0, r1, kbs in REGIONS:
                            if kb not in kbs:
                                continue
                            nc.tensor.matmul(
                                out_ps[:, mi * 256 + r0 : mi * 256 + r1],
                                lhsT,
                                rhs_t[:, kb * 256 + r0 : kb * 256 + r1],
                                start=(kb == kbs[0]),
                                stop=(kb == kbs[-1]),
                            )

            # Two-stage software pipeline with a DEPTH-slab stagger: MM2 for
            # slab i is emitted after MM1 for slab i+DEPTH.  PE executes in
            # program order, so without the stagger it would idle between
            # MM1(i) and MM2(i) while VectorE drains pv(i) into vt(i); with
            # it, PE streams MM1s of later slabs during every copy.
            def batch_loop():
                pending = []  # (vt, q, slab)
                zs_box = [None]

                def flush_one():
                    vt, q, slab = pending.pop(0)
                    # MM2: Z[h', w'] = sum_w V^T[w, h'] * B[w, w']
                    pz = pzpool.tile([128, 512], f32)
                    emit_mm(
                        pz,
                        lambda mi, kb: vt[
                            :, kb * 256 + mi * 128 : kb * 256 + mi * 128 + 128
                        ],
                        b_t[q],
                    )
                    go = slab % g_out
                    if go == 0:
                        zs_box[0] = zspool.tile(
                            [128, g_out * 512], bf16, tag="zs", name="zs"
                        )
                    zs = zs_box[0]
                    nc.scalar.copy(zs[:, go * 512 : (go + 1) * 512], pz[:])
                    if go == g_out - 1:
                        out_dma_eng.dma_start(o_d[slab // g_out], zs[:])

                for ch in range(NCH_IN):
                    ld = ldpool.tile([128, G_IN * 512], bf16)
                    in_dma_eng.dma_start(ld[:], u_d[ch])
                    for g in range(G_IN):
                        slab = ch * G_IN + g
                        q = 0 if slab < 32 else 1
                        base = g * 512
                        # MM1: V^T[w, h'] = sum_h U[h, w] * A^T[h, h']
                        pv = pvpool.tile([128, 512], f32)
                        emit_mm(
                            pv,
                            lambda mi, kb, base=base, ld=ld: ld[
                                :,
                                base + kb * 256 + mi * 128 : base + kb * 256 + mi * 128 + 128,
                            ],
                            a_t[q],
                        )
                        vt = vtpool.tile([128, 512], bf16)
                        nc.vector.tensor_copy(vt[:], pv[:])
                        pending.append((vt, q, slab))
                        while len(pending) > cfg["depth"]:
                            flush_one()
                while pending:
                    flush_one()

            if repeat == 1:
                batch_loop()
            else:
                # staggered_reset avoids the ~3us all-engine barrier at the
                # loop back-edge, so the slope measurement better matches the
                # barrier-free single-shot kernel.
                with tc.For_i(0, repeat, 1, staggered_reset=True):
                    batch_loop()
    nc.compile()
    return nc


_CACHE = {}


def _axon_runner():
    """Build (once) a jitted 8-way sharded executor for the axon/PJRT path.

    Mirrors concourse.bass2jax.run_bass_via_pjrt but keeps the compiled
    executable alive so repeat kernel() calls skip retracing + NEFF
    recompilation.
    """
    if "runner" in _CACHE:
        return _CACHE["runner"]
    import jax
    from jax.experimental.shard_map import shard_map
    from jax.sharding import Mesh, NamedSharding, PartitionSpec

    from concourse import bass2jax, mybir

    nc = build_module()
    bass2jax.install_neuronx_cc_hook()
    partition_name = nc.partition_id_tensor.name if nc.partition_id_tensor else None
    in_names, out_names, out_avals = [], [], []
    for alloc in nc.m.functions[0].allocations:
        if not isinstance(alloc, mybir.MemoryLocationSet):
            continue
        name = alloc.memorylocations[0].name
        if alloc.kind == "ExternalInput":
            if name != partition_name:
                in_names.append(name)
        elif alloc.kind == "ExternalOutput":
            out_names.append(name)
            out_avals.append(
                jax.core.ShapedArray(tuple(alloc.tensor_shape), mybir.dt.np(alloc.dtype))
            )
    n_params = len(in_names)
    n_outs = len(out_avals)
    all_names = in_names + out_names + ([partition_name] if partition_name else [])
    donate = tuple(range(n_params, n_params + n_outs))

    def _body(*args):
        operands = list(args)
        if partition_name is not None:
            operands.append(bass2jax.partition_id_tensor())
        return tuple(
            bass2jax._bass_exec_p.bind(
                *operands,
                out_avals=tuple(out_avals),
                in_names=tuple(all_names),
                out_names=tuple(out_names),
                lowering_input_output_aliases=(),
                sim_require_finite=True,
                sim_require_nnan=True,
                nc=nc,
            )
        )

    devices = jax.devices()[:N_CORES]
    mesh = Mesh(np.asarray(devices), ("core",))
    spec = NamedSharding(mesh, PartitionSpec("core"))
    sharded = jax.jit(
        shard_map(
            _body,
            mesh=mesh,
            in_specs=(PartitionSpec("core"),) * (n_params + n_outs),
            out_specs=(PartitionSpec("core"),) * n_outs,
            check_rep=False,
        ),
        donate_argnums=donate,
        keep_unused=True,
    )

    def run(u_cores, mats_cores):
        per_core = {
            "u": np.concatenate(u_cores, axis=0),
            "mats": np.concatenate(mats_cores, axis=0),
        }
        xs = [jax.device_put(per_core[nm], spec) for nm in in_names]
        zs = [
            jax.device_put(
                np.zeros((N_CORES * a.shape[0], *a.shape[1:]), a.dtype), spec
            )
            for a in out_avals
        ]
        outs = sharded(*xs, *zs)
        out = np.asarray(outs[out_names.index("out")])
        return out.reshape(N_CORES, NCH_OUT, 128, G_OUT * 512)

    _CACHE["runner"] = run
    return run


def kernel(u, alpha_base, beta_base, alpha_spatial, beta_spatial, channel_coupling):
    from concourse._compat import axon_active

    u = np.ascontiguousarray(np.asarray(u, dtype=np.float32))
    mats_full = _host_mats(
        np.asarray(alpha_base, dtype=np.float32),
        np.asarray(beta_base, dtype=np.float32),
        np.asarray(alpha_spatial, dtype=np.float32),
        np.asarray(beta_spatial, dtype=np.float32),
        np.asarray(channel_coupling, dtype=np.float32),
    )
    u_cores, mats_cores, idxs = prep_core_arrays(u, mats_full)

    if axon_active():
        res = _axon_runner()(u_cores, mats_cores)
    else:
        # Native path (/dev/neuron* present): run via NRT on cores 0-7.
        from concourse.bass_utils import run_bass_kernel_spmd

        nc = _CACHE.setdefault("nc", build_module())
        in_maps = [
            {"u": u_cores[k], "mats": mats_cores[k]} for k in range(N_CORES)
        ]
        rr = run_bass_kernel_spmd(nc, in_maps, core_ids=list(range(N_CORES)))
        res = np.stack([r["out"] for r in rr.results])

    out = np.empty((B, C, S, S), dtype=np.float32)
    for k in range(N_CORES):
        b_idx, c_idx = idxs[k]
        out[b_idx, c_idx] = _unpack_out_core(res[k])
    return out


# revision 28
# speedup vs baseline: 2.0535x; 1.0193x over previous
"""Trainium2 Bass kernel for nn_CIFARDiffusionLayer.

The reference applies, per channel c, three ADI steps; each step is an
x-sweep (constant-coefficient tridiagonal solve along W), a y-sweep
(same along H), and a multiply by diag(channel_coupling)[c].  Every
sweep is a fixed linear map, so the whole layer collapses to

    out[b, c] = A_c @ u[b, c] @ B_c
    A_c = s_c^3 * My(c,2) @ My(c,1) @ My(c,0)      (s_c = coupling diag)
    B_c = Mx(c,0)^T @ Mx(c,1)^T @ Mx(c,2)^T

with the 256x256 matrices computed on the host in float64 from the
reference's exact recurrences (including its eps quirks).

Device-side optimizations over a dense-fp32 version:
  * Narrow dtypes end-to-end: the input is quantized per slab to int8 on
    the host (per-slab absmax scales, folded back out on the host after
    the run since the pipeline is linear per slab) and upcast to bf16 by
    the SWDGE DMA cast in flight; matrices, intermediate, and output are
    bf16.  HBM traffic drops from ~25 MB/core (fp32) to ~9.4 MB/core,
    which is the roofline for this memory-bound problem; quantization +
    bf16 rounding contribute ~1.0e-2 relative error against the 2e-2
    budget.
  * The matrices are inverses of tridiagonal systems with r~0.0075, so
    their entries decay by ~2e-2 per step off the diagonal: beyond
    |i-j| >= 16 they are exactly zero in fp32.  Each 256-column matmul
    contraction is split into disjoint output regions [0,112) (k-block
    0 only), [112,144) (both blocks), [144,256) (block 1 only), cutting
    PE streaming from 1024 to 576 columns per slab-side.
  * Host pre-packs inputs into the exact SBUF layout ([partition, k, w]
    interleave) so every DMA is a flat 128-partition transfer with
    multi-KB contiguous runs per partition (1 MB input chunks of 8
    slabs, 512 KB output chunks of 4 slabs).
  * Two-stage software pipeline: MM2 of slab i is emitted after MM1 of
    slab i+4, so TensorE (in-order execution) never stalls behind the
    PSUM->SBUF drain of the intermediate; VectorE handles that drain
    while ScalarE stages outputs.  In/out DMAs both ride the SP HWDGE
    ring (measured faster than ACT-issued out-DMAs, whose sem-wait
    blocks the ACT sequencer between copy bursts).

Sharding: data parallelism over (batch, channel) slabs: 384 slabs are
dealt to 8 cores as 48 slabs each (32 of one channel + 16 of another,
per the ASSIGN table), so each core loads only the 2 matrix pairs it
needs while the NEFF stays identical across cores.
"""

import sys

if "/opt/trn_rl_repo" not in sys.path:
    sys.path.insert(0, "/opt/trn_rl_repo")

import numpy as np
import ml_dtypes

BF16 = ml_dtypes.bfloat16

DT = 0.05
DX = 1.0
NUM_STEPS = 3
EPS = 1e-6
MAX_COEFF = 1.0

N_CORES = 8
B, C, S = 128, 3, 256
N_SLAB = 48          # (batch, channel) slabs per core
G_IN = 8             # slabs per input DMA chunk
G_OUT = 4            # slabs per output DMA chunk
NCH_IN = N_SLAB // G_IN
NCH_OUT = N_SLAB // G_OUT
# Banded contraction: output regions and which 128-row k-blocks feed them.
REGIONS = [(0, 112, (0,)), (112, 144, (0, 1)), (144, 256, (1,))]

# Per core: ((channel of the 32-slab block, batch start), (channel of the
# 16-slab block, batch start)).  Covers each (b, c) exactly once:
# c0 = 4x32, c1 = 2x32 + 4x16, c2 = 2x32 + 4x16.
ASSIGN = [
    ((0, 0), (1, 64)),
    ((0, 32), (1, 80)),
    ((0, 64), (1, 96)),
    ((0, 96), (1, 112)),
    ((1, 0), (2, 64)),
    ((1, 32), (2, 80)),
    ((2, 0), (2, 96)),
    ((2, 32), (2, 112)),
]


def _core_slab_indices(k):
    (c32, b32), (c16, b16) = ASSIGN[k]
    b_idx = list(range(b32, b32 + 32)) + list(range(b16, b16 + 16))
    c_idx = [c32] * 32 + [c16] * 16
    return b_idx, c_idx


def _thomas_inv(r: float, n: int = S, eps: float = EPS) -> np.ndarray:
    """T^{-1} for the reference's constant-coefficient Thomas solve.

    Mirrors reference._thomas_const exactly (b[0]+eps on the first
    denominator, clamp(min=eps) on interior denominators), evaluated in
    float64 on the identity RHS so columns are T^{-1} e_j.
    """
    a = -r
    b = np.full(n, 1.0 + 2.0 * r, dtype=np.float64)
    b[0] = b[-1] = 1.0 + r
    denom = np.empty(n, dtype=np.float64)
    cp = np.empty(n, dtype=np.float64)
    denom[0] = b[0] + eps
    cp[0] = a / denom[0]
    for i in range(1, n):
        denom[i] = max(b[i] - a * cp[i - 1], eps)
        cp[i] = a / denom[i]
    dp = np.zeros((n, n), dtype=np.float64)
    eye = np.eye(n, dtype=np.float64)
    dp[0] = eye[0] / denom[0]
    for i in range(1, n):
        dp[i] = (eye[i] - a * dp[i - 1]) / denom[i]
    x = np.zeros((n, n), dtype=np.float64)
    x[n - 1] = dp[n - 1]
    for i in range(n - 2, -1, -1):
        x[i] = dp[i] - cp[i] * x[i + 1]
    return x


def _host_mats(alpha_base, beta_base, alpha_spatial, beta_spatial, channel_coupling):
    """mats[c, 0] = A_c^T, mats[c, 1] = B_c, as float32 [C, 2, S, S]."""
    diag = np.diagonal(np.asarray(channel_coupling)).astype(np.float64)
    mats = np.empty((C, 2, S, S), dtype=np.float32)
    for c in range(C):
        am = float(np.mean(np.asarray(alpha_spatial[c], dtype=np.float64)))
        bm = float(np.mean(np.asarray(beta_spatial[c], dtype=np.float64)))
        a_c = np.eye(S, dtype=np.float64)
        b_c = np.eye(S, dtype=np.float64)
        for step in range(NUM_STEPS):
            t = step * DT
            alpha_t = min(max(float(alpha_base[c]) + am * t, EPS), MAX_COEFF)
            beta_t = min(max(float(beta_base[c]) + bm * t, EPS), MAX_COEFF)
            r_a = alpha_t * (DT / 2.0) / DX**2
            r_b = beta_t * (DT / 2.0) / DX**2
            a_c = _thomas_inv(r_b) @ a_c
            b_c = b_c @ _thomas_inv(r_a).T
        mats[c, 0] = (diag[c] ** 3 * a_c).T.astype(np.float32)
        mats[c, 1] = b_c.astype(np.float32)
    return mats


INT8_IN = True   # int8 input + SWDGE int8->bf16 cast during DMA (validated on HW)


def _pack_u_core(u_core, int8_in=None):
    """[48, 256, 256] f32 -> [NCH_IN, 128, G_IN*512] device dtype, [p, g, k, w].

    With int8_in, u is symmetrically quantized to int8 per slab (the per-slab
    scales are returned; the host divides each output slab by its scale after
    unpacking); the input DMA upcasts to bf16 on the fly (SWDGE cast),
    halving input HBM traffic.
    """
    if int8_in is None:
        int8_in = INT8_IN
    if int8_in:
        # Per-slab absmax scaling: no clipping, and since the whole pipeline
        # is linear per slab, the host just divides each output slab by its
        # scale after unpacking.  The int values are exactly representable in
        # the bf16 the DMA casts to.
        scale = 127.0 / (np.abs(u_core).max(axis=(1, 2), keepdims=True) + 1e-30)
        q = np.rint(u_core * scale).astype(np.int8)
        x = q.reshape(NCH_IN, G_IN, 2, 128, 256)
        scale = scale.reshape(N_SLAB)
    else:
        scale = None
        x = u_core.astype(BF16).reshape(NCH_IN, G_IN, 2, 128, 256)
    x = np.ascontiguousarray(x.transpose(0, 3, 1, 2, 4))
    return x.reshape(NCH_IN, 128, G_IN * 512), scale


def _pack_mats_core(mats_core):
    """[2, 2, 256, 256] f32 -> [2, 2, 128, 512] bf16 in [p, k, col] order."""
    m = mats_core.astype(BF16).reshape(2, 2, 2, 128, 256)
    m = np.ascontiguousarray(m.transpose(0, 1, 3, 2, 4))
    return m.reshape(2, 2, 128, 512)


def _unpack_out_core(r):
    """[NCH_OUT, 128, G_OUT*512] bf16 -> [48, 256, 256] f32."""
    x = r.reshape(NCH_OUT, 128, G_OUT, 2, 256).transpose(0, 2, 3, 1, 4)
    return np.ascontiguousarray(x).reshape(N_SLAB, S, S).astype(np.float32)


def prep_core_arrays(u, mats_full, int8_in=None):
    """Returns (u_cores, mats_cores, idxs, scales): packed per-core inputs.

    scales[k] is the per-slab quant scale array (or None for bf16 input);
    the caller divides each unpacked output slab by its scale.
    """
    u_cores, mats_cores, idxs, scales = [], [], [], []
    for k in range(N_CORES):
        b_idx, c_idx = _core_slab_indices(k)
        idxs.append((b_idx, c_idx))
        uc, scale = _pack_u_core(np.ascontiguousarray(u[b_idx, c_idx]), int8_in)
        u_cores.append(uc)
        scales.append(scale)
        (c32, _), (c16, _) = ASSIGN[k]
        mats_cores.append(
            _pack_mats_core(np.stack([mats_full[c32], mats_full[c16]]))
        )
    return u_cores, mats_cores, idxs, scales


def build_module(repeat: int = 1, **overrides):
    """Per-core Bass module: out[b,c] = A_c @ u[b,c] @ B_c for 48 slabs.

    repeat > 1 wraps the batch loop in a hardware For_i that re-runs the
    whole kernel body; only used by the timing harness (wall-clock slope
    between two repeat counts isolates the per-iteration device time).
    overrides: tuning knobs for the HW sweep harness (see cfg below).
    """
    import concourse.bacc as bacc
    import concourse.tile as tile
    from concourse import mybir

    cfg = dict(
        depth=4, ldb=3, vtb=6, zsb=4, pvb=4, pzb=4, out_eng="sp", in_eng="sp",
        g_out=G_OUT, int8_in=INT8_IN,
    )
    cfg.update(overrides)
    g_out = cfg["g_out"]
    if cfg["int8_in"]:
        cfg["in_eng"] = "gp"  # only SWDGE can cast during DMA

    f32, bf16 = mybir.dt.float32, mybir.dt.bfloat16
    nc = bacc.Bacc(
        "TRN2",
        target_bir_lowering=False,
        debug=False,
        enable_asserts=False,
        num_devices=N_CORES,
    )
    u_dt = mybir.dt.int8 if cfg["int8_in"] else bf16
    u_d = nc.dram_tensor("u", [NCH_IN, 128, G_IN * 512], u_dt, kind="ExternalInput")
    m_d = nc.dram_tensor("mats", [2, 2, 128, 512], bf16, kind="ExternalInput")
    o_d = nc.dram_tensor("out", [N_SLAB // g_out, 128, g_out * 512], bf16, kind="ExternalOutput")

    with tile.TileContext(nc) as tc:
        with (
            tc.tile_pool(name="consts", bufs=1) as cpool,
            tc.tile_pool(name="ld", bufs=cfg["ldb"]) as ldpool,
            tc.tile_pool(name="vt", bufs=cfg["vtb"]) as vtpool,
            tc.tile_pool(name="zs", bufs=cfg["zsb"]) as zspool,
            tc.tile_pool(name="pv", bufs=cfg["pvb"], space="PSUM") as pvpool,
            tc.tile_pool(name="pz", bufs=cfg["pzb"], space="PSUM") as pzpool,
        ):
            engs = {"sp": nc.sync, "act": nc.scalar, "gp": nc.gpsimd}
            out_dma_eng = engs[cfg["out_eng"]]
            in_dma_eng = engs[cfg["in_eng"]]
            # Matrix pair q in {0,1}; [128, 512] = [h%128, (h//128)*256 + col].
            # Const loads ride the ACT HWDGE ring, which is otherwise idle at
            # start, so the first input chunk on the SP ring is not delayed.
            a_t, b_t = [], []
            for q in range(2):
                at = cpool.tile([128, 512], bf16, tag=f"a{q}")
                nc.scalar.dma_start(at[:], m_d[q, 0])
                a_t.append(at)
                bt = cpool.tile([128, 512], bf16, tag=f"b{q}")
                nc.scalar.dma_start(bt[:], m_d[q, 1])
                b_t.append(bt)

            def emit_mm(out_ps, lhs_of, rhs_t):
                """One 256x256x256 product as banded region matmuls.

                lhs_of(mi, kb) yields the stationary [128,128] data tile;
                rhs_t is the [128, 512] matrix tile ([row%128, kb*256+col]).
                """
                for mi in range(2):
                    for kb in range(2):
                        lhsT = lhs_of(mi, kb)
                        for r0, r1, kbs in REGIONS:
                            if kb not in kbs:
                                continue
                            nc.tensor.matmul(
                                out_ps[:, mi * 256 + r0 : mi * 256 + r1],
                                lhsT,
                                rhs_t[:, kb * 256 + r0 : kb * 256 + r1],
                                start=(kb == kbs[0]),
                                stop=(kb == kbs[-1]),
                            )

            # Two-stage software pipeline with a DEPTH-slab stagger: MM2 for
            # slab i is emitted after MM1 for slab i+DEPTH.  PE executes in
            # program order, so without the stagger it would idle between
            # MM1(i) and MM2(i) while VectorE drains pv(i) into vt(i); with
            # it, PE streams MM1s of later slabs during every copy.
            def batch_loop():
                pending = []  # (vt, q, slab)
                zs_box = [None]

                def flush_one():
                    vt, q, slab = pending.pop(0)
                    # MM2: Z[h', w'] = sum_w V^T[w, h'] * B[w, w']
                    pz = pzpool.tile([128, 512], f32)
                    emit_mm(
                        pz,
                        lambda mi, kb: vt[
                            :, kb * 256 + mi * 128 : kb * 256 + mi * 128 + 128
                        ],
                        b_t[q],
                    )
                    go = slab % g_out
                    if go == 0:
                        zs_box[0] = zspool.tile(
                            [128, g_out * 512], bf16, tag="zs", name="zs"
                        )
                    zs = zs_box[0]
                    nc.scalar.copy(zs[:, go * 512 : (go + 1) * 512], pz[:])
                    if go == g_out - 1:
                        out_dma_eng.dma_start(o_d[slab // g_out], zs[:])

                for ch in range(NCH_IN):
                    ld = ldpool.tile([128, G_IN * 512], bf16)
                    in_dma_eng.dma_start(ld[:], u_d[ch])
                    for g in range(G_IN):
                        slab = ch * G_IN + g
                        q = 0 if slab < 32 else 1
                        base = g * 512
                        # MM1: V^T[w, h'] = sum_h U[h, w] * A^T[h, h']
                        pv = pvpool.tile([128, 512], f32)
                        emit_mm(
                            pv,
                            lambda mi, kb, base=base, ld=ld: ld[
                                :,
                                base + kb * 256 + mi * 128 : base + kb * 256 + mi * 128 + 128,
                            ],
                            a_t[q],
                        )
                        vt = vtpool.tile([128, 512], bf16)
                        nc.vector.tensor_copy(vt[:], pv[:])
                        pending.append((vt, q, slab))
                        while len(pending) > cfg["depth"]:
                            flush_one()
                while pending:
                    flush_one()

            if repeat == 1:
                batch_loop()
            else:
                # staggered_reset avoids the ~3us all-engine barrier at the
                # loop back-edge, so the slope measurement better matches the
                # barrier-free single-shot kernel.
                with tc.For_i(0, repeat, 1, staggered_reset=True):
                    batch_loop()
    nc.compile()
    return nc


_CACHE = {}


def _axon_runner():
    """Build (once) a jitted 8-way sharded executor for the axon/PJRT path.

    Mirrors concourse.bass2jax.run_bass_via_pjrt but keeps the compiled
    executable alive so repeat kernel() calls skip retracing + NEFF
    recompilation.
    """
    if "runner" in _CACHE:
        return _CACHE["runner"]
    import jax
    from jax.experimental.shard_map import shard_map
    from jax.sharding import Mesh, NamedSharding, PartitionSpec

    from concourse import bass2jax, mybir

    nc = build_module()
    bass2jax.install_neuronx_cc_hook()
    partition_name = nc.partition_id_tensor.name if nc.partition_id_tensor else None
    in_names, out_names, out_avals = [], [], []
    for alloc in nc.m.functions[0].allocations:
        if not isinstance(alloc, mybir.MemoryLocationSet):
            continue
        name = alloc.memorylocations[0].name
        if alloc.kind == "ExternalInput":
            if name != partition_name:
                in_names.append(name)
        elif alloc.kind == "ExternalOutput":
            out_names.append(name)
            out_avals.append(
                jax.core.ShapedArray(tuple(alloc.tensor_shape), mybir.dt.np(alloc.dtype))
            )
    n_params = len(in_names)
    n_outs = len(out_avals)
    all_names = in_names + out_names + ([partition_name] if partition_name else [])
    donate = tuple(range(n_params, n_params + n_outs))

    def _body(*args):
        operands = list(args)
        if partition_name is not None:
            operands.append(bass2jax.partition_id_tensor())
        return tuple(
            bass2jax._bass_exec_p.bind(
                *operands,
                out_avals=tuple(out_avals),
                in_names=tuple(all_names),
                out_names=tuple(out_names),
                lowering_input_output_aliases=(),
                sim_require_finite=True,
                sim_require_nnan=True,
                nc=nc,
            )
        )

    devices = jax.devices()[:N_CORES]
    mesh = Mesh(np.asarray(devices), ("core",))
    spec = NamedSharding(mesh, PartitionSpec("core"))
    sharded = jax.jit(
        shard_map(
            _body,
            mesh=mesh,
            in_specs=(PartitionSpec("core"),) * (n_params + n_outs),
            out_specs=(PartitionSpec("core"),) * n_outs,
            check_rep=False,
        ),
        donate_argnums=donate,
        keep_unused=True,
    )

    def run(u_cores, mats_cores):
        per_core = {
            "u": np.concatenate(u_cores, axis=0),
            "mats": np.concatenate(mats_cores, axis=0),
        }
        xs = [jax.device_put(per_core[nm], spec) for nm in in_names]
        zs = [
            jax.device_put(
                np.zeros((N_CORES * a.shape[0], *a.shape[1:]), a.dtype), spec
            )
            for a in out_avals
        ]
        outs = sharded(*xs, *zs)
        out = np.asarray(outs[out_names.index("out")])
        return out.reshape(N_CORES, NCH_OUT, 128, G_OUT * 512)

    _CACHE["runner"] = run
    return run


def kernel(u, alpha_base, beta_base, alpha_spatial, beta_spatial, channel_coupling):
    from concourse._compat import axon_active

    u = np.ascontiguousarray(np.asarray(u, dtype=np.float32))
    mats_full = _host_mats(
        np.asarray(alpha_base, dtype=np.float32),
        np.asarray(beta_base, dtype=np.float32),
        np.asarray(alpha_spatial, dtype=np.float32),
        np.asarray(beta_spatial, dtype=np.float32),
        np.asarray(channel_coupling, dtype=np.float32),
    )
    u_cores, mats_cores, idxs, scales = prep_core_arrays(u, mats_full)

    if axon_active():
        res = _axon_runner()(u_cores, mats_cores)
    else:
        # Native path (/dev/neuron* present): run via NRT on cores 0-7.
        from concourse.bass_utils import run_bass_kernel_spmd

        nc = _CACHE.setdefault("nc", build_module())
        in_maps = [
            {"u": u_cores[k], "mats": mats_cores[k]} for k in range(N_CORES)
        ]
        rr = run_bass_kernel_spmd(nc, in_maps, core_ids=list(range(N_CORES)))
        res = np.stack([r["out"] for r in rr.results])

    out = np.empty((B, C, S, S), dtype=np.float32)
    for k in range(N_CORES):
        b_idx, c_idx = idxs[k]
        oc = _unpack_out_core(res[k])
        if scales[k] is not None:
            oc /= scales[k][:, None, None].astype(np.float32)
        out[b_idx, c_idx] = oc
    return out
